# revision 1
# baseline (speedup 1.0000x reference)
"""Trainium2 Bass kernel for a 2-layer GCN encoder + global mean pool.

Problem: x[100000,128] f32, edge_index[2,1600000] i64, batch[100000] i64
(sorted), W1/b1/W2/b2. Two GCNConv layers (symmetric deg^-1/2 norm, self
loops, relu) then mean-pool over 512 graphs -> [512, 128] f32.

Strategy (8 NeuronCores, data-parallel over graphs):
- Nodes partitioned by graph id into 8 contiguous shards (batch is sorted);
  each core owns the edges whose *destination* lands in its shard.
- Algebraic rewrite: aggregate-then-transform.  For each layer,
      h' = relu( (A_hat @ h) @ W + b ),   A_hat = D^-1/2 (A+I) D^-1/2
  so the sparse aggregation runs on raw 128-dim features and the dense
  matmul with W happens per 128-node window afterwards.  Self loops are
  plain edges with weight 1/deg.
- Sparse aggregation per core: per 128-edge block, dma_gather fetches the
  128 source rows (bf16, 256B each) from the feature table; a one-hot
  selection matrix M (built on the Vector engine from precomputed dst-slot
  and edge-weight columns) scatters them on the Tensor engine:
      PSUM[f, s] += G_blk.T @ M_blk
  accumulated over all blocks of a 128-node destination window.
- dma_gather uses int16 indices, so the node table is addressed through
  <=32768-row "quarter" slices; edges are sorted by (window-group, quarter,
  window) so each gather call stays quarter-pure while the PSUM tiles of a
  4-window group stay resident across the group's quarter runs.
- The finished PSUM window (= Z^T) feeds the dense W matmul directly (its
  transposed layout is exactly the lhsT the Tensor engine wants); bias is
  added via a K=1 matmul with a ones row; relu on the Scalar engine.
- Between layers one 8-core AllGather shares the per-shard h1 table (bf16).
- Mean pool: the same one-hot matmul trick keyed on local graph id into a
  single resident PSUM bank, then a reciprocal-count scale.

Host-side preprocessing is purely structural (index sorting, degree counts,
normalization coefficients derived from the graph topology); all
feature/weight compute runs on device.
"""

import math
import os

import numpy as np
import ml_dtypes

import concourse.bass as bass
import concourse.bacc as bacc
import concourse.mybir as mybir
import concourse.tile as tile
from concourse.bass_utils import run_bass_kernel_spmd

P = 128
C = 8               # cores
G = 512             # graphs
GPC = G // C        # graphs per core
F = 128             # feature dim (in = hid = out)
WG = 4              # windows per PSUM-resident group
QROWS = 32768       # rows per int16-addressable table slice
CALLBLK = 8         # max edge blocks per dma_gather call (1024 idx;
                    # larger single calls crash the SWDGE ucode)

bf16 = mybir.dt.bfloat16
f32 = mybir.dt.float32
i16 = mybir.dt.int16

BF = ml_dtypes.bfloat16


def _preprocess(x, edge_index, batch):
    """Structural preprocessing: shard nodes by graph, sort/pad edges by
    (window-group, src-quarter, dst-window), compute GCN norm weights."""
    N = x.shape[0]
    src = np.asarray(edge_index[0], dtype=np.int64)
    dst = np.asarray(edge_index[1], dtype=np.int64)
    batch = np.asarray(batch, dtype=np.int64)

    node_start = np.searchsorted(batch, np.arange(C + 1) * GPC).astype(np.int64)
    nk = np.diff(node_start)
    NODE_PAD = int(math.ceil(nk.max() / P) * P)
    NW = NODE_PAD // P
    TOT = C * NODE_PAD
    assert TOT <= 4 * QROWS
    NG = (NW + WG - 1) // WG

    core_of = (batch // GPC).astype(np.int64)
    row = (np.arange(N) - node_start[core_of] + core_of * NODE_PAD).astype(np.int64)

    deg = np.bincount(dst, minlength=N).astype(np.float64) + 1.0
    dis = 1.0 / np.sqrt(deg)

    # full edge list including self loops
    esrc = np.concatenate([src, np.arange(N)])
    edst = np.concatenate([dst, np.arange(N)])
    ew = np.concatenate([dis[src] * dis[dst], 1.0 / deg]).astype(np.float32)

    ecore = core_of[edst]
    eld = edst - node_start[ecore]
    ewin = eld >> 7
    eslot = (eld & 127).astype(np.float32)
    esrcrow = row[esrc]
    eq = (esrcrow // QROWS).astype(np.int64)      # source quarter
    eloc = (esrcrow % QROWS).astype(np.int64)     # quarter-local row
    egrp = ewin // WG

    NQ = 4
    # segment id in (core, group, quarter, window) order
    seg = ((ecore * NG + egrp) * NQ + eq) * NW + ewin
    NSEG = C * NG * NQ * NW
    counts = np.bincount(seg, minlength=NSEG)
    cnt4 = counts.reshape(C, NG, NQ, NW)
    # SPMD-common block counts per (group, quarter, window)
    BWS = np.ceil(cnt4.max(axis=0) / P).astype(np.int64)      # [NG, NQ, NW]
    for g in range(NG):
        mask = np.zeros(NW, bool)
        mask[g * WG:(g + 1) * WG] = True
        BWS[g, :, ~mask] = 0
    # every window needs >= 1 block so its PSUM tile is always written
    for w in range(NW):
        g = w // WG
        if BWS[g, :, w].sum() == 0:
            BWS[g, 0, w] = 1
    NBLK = int(BWS.sum())

    # block/segment offsets in (g, q, w) order
    seg_order = []          # (g, q, w, block_start, nblocks)
    seg_start = np.zeros((NG, NQ, NW), np.int64)
    acc = 0
    for g in range(NG):
        for q in range(NQ):
            for w in range(g * WG, min((g + 1) * WG, NW)):
                seg_start[g, q, w] = acc
                nb = int(BWS[g, q, w])
                if nb:
                    seg_order.append((g, q, w, acc, nb))
                acc += nb
    assert acc == NBLK

    # scatter edges into the padded per-core layout
    order = np.argsort(seg, kind="stable")
    seg_sorted = seg[order]
    grp_excl = np.concatenate([[0], np.cumsum(counts)[:-1]])
    pos = np.arange(order.size) - grp_excl[seg_sorted]
    es = order
    dest = (ecore[es] * (NBLK * P)
            + seg_start[egrp[es], eq[es], ewin[es]] * P + pos)

    idx_arr = np.zeros(C * NBLK * P, np.int16)      # quarter-local src row
    slot_arr = np.zeros(C * NBLK * P, np.float32)
    w_arr = np.zeros(C * NBLK * P, np.float32)
    idx_arr[dest] = eloc[es].astype(np.int16)
    slot_arr[dest] = eslot[es]
    w_arr[dest] = ew[es]

    # per-core uploads
    # idx: wrapped [16, NBLK*8] (logical i at [i%16, i//16]), replicated to
    # 128 partitions (the gather ucode's per-Q7-core channel groups all read
    # the same wrap)
    idx_pc = np.ascontiguousarray(
        idx_arr.reshape(C, NBLK * P // 16, 16).transpose(0, 2, 1))
    idx_pc = np.ascontiguousarray(np.tile(idx_pc, (1, 8, 1)))
    slot_pc = np.ascontiguousarray(slot_arr.reshape(C, NBLK, P).transpose(0, 2, 1))
    w_pc = np.ascontiguousarray(w_arr.reshape(C, NBLK, P).transpose(0, 2, 1))

    # node feature table, padded/bf16
    xt = np.zeros((TOT, F), BF)
    xt[row] = np.asarray(x, np.float32).astype(BF)

    # static schedule: per block -> (window, first/last-of-window);
    # gather calls: chunks of <= CALLBLK blocks within one (g, q) run.
    blk_win = np.zeros(NBLK, np.int64)
    first_blk = {}
    last_blk = {}
    for (g, q, w, b0, nb) in seg_order:
        blk_win[b0:b0 + nb] = w
        if w not in first_blk:
            first_blk[w] = b0
        last_blk[w] = b0 + nb - 1
    blk_first = np.zeros(NBLK, bool)
    blk_last = np.zeros(NBLK, bool)
    for w, b in first_blk.items():
        blk_first[b] = True
    for w, b in last_blk.items():
        blk_last[b] = True

    calls = []   # (b0, nb, quarter)
    run_key = None
    run_blocks = []
    runs = []
    for (g, q, w, b0, nb) in seg_order:
        if (g, q) != run_key:
            if run_blocks:
                runs.append((run_key[1], run_blocks))
            run_key = (g, q)
            run_blocks = []
        run_blocks.append((b0, nb))
    if run_blocks:
        runs.append((run_key[1], run_blocks))
    for q, blocks in runs:
        b0 = blocks[0][0]
        bend = blocks[-1][0] + blocks[-1][1]
        b = b0
        while b < bend:
            nb = min(CALLBLK, bend - b)
            calls.append((b, nb, q))
            b += nb

    # pooling metadata
    batloc = np.full((C, NODE_PAD), -1.0, np.float32)
    for c in range(C):
        nn = int(nk[c])
        batloc[c, :nn] = (batch[node_start[c]:node_start[c + 1]] - c * GPC).astype(
            np.float32)
    batloc_pc = np.ascontiguousarray(batloc.reshape(C, NW, P).transpose(0, 2, 1))

    gcnt = np.bincount(batch, minlength=G).astype(np.float32)
    counts_pc = np.ones((C, P, 1), np.float32)
    counts_pc[:, :GPC, 0] = gcnt.reshape(C, GPC)

    return dict(
        NODE_PAD=NODE_PAD, NW=NW, TOT=TOT, NBLK=NBLK,
        blk_win=blk_win, blk_first=blk_first, blk_last=blk_last, calls=calls,
        idx_pc=idx_pc, slot_pc=slot_pc, w_pc=w_pc, xt=xt,
        batloc_pc=batloc_pc, counts_pc=counts_pc,
    )


def _build_nc(pre):
    NW = pre["NW"]
    NBLK = pre["NBLK"]
    TOT = pre["TOT"]
    NODE_PAD = pre["NODE_PAD"]
    blk_win = pre["blk_win"]
    blk_first = pre["blk_first"]
    blk_last = pre["blk_last"]
    calls = pre["calls"]

    _nq = int(os.environ.get("KERNEL_NQUEUES", "4"))
    nc = bacc.Bacc(None, num_devices=C, num_swdge_queues=_nq)

    xt_d = nc.dram_tensor("xt", [TOT, F], bf16, kind="ExternalInput")
    idx_d = nc.dram_tensor("eidx", [128, NBLK * 8], i16, kind="ExternalInput")
    slot_d = nc.dram_tensor("eslot", [P, NBLK], f32, kind="ExternalInput")
    ew_d = nc.dram_tensor("ew", [P, NBLK], f32, kind="ExternalInput")
    iota_d = nc.dram_tensor("iota", [P, P], bf16, kind="ExternalInput")
    ones_d = nc.dram_tensor("ones", [1, P], bf16, kind="ExternalInput")
    w1_d = nc.dram_tensor("w1", [F, F], bf16, kind="ExternalInput")
    w2_d = nc.dram_tensor("w2", [F, F], bf16, kind="ExternalInput")
    b1_d = nc.dram_tensor("b1", [1, F], bf16, kind="ExternalInput")
    b2_d = nc.dram_tensor("b2", [1, F], bf16, kind="ExternalInput")
    batloc_d = nc.dram_tensor("batloc", [P, NW], f32, kind="ExternalInput")
    cnts_d = nc.dram_tensor("cnts", [P, 1], f32, kind="ExternalInput")
    out_d = nc.dram_tensor("out", [GPC, F], f32, kind="ExternalOutput")

    with tile.TileContext(nc) as tc:
        with (
            tc.tile_pool(name="const", bufs=1) as cpool,
            tc.tile_pool(name="gbuf", bufs=6) as gpool,
            tc.tile_pool(name="mt", bufs=4) as mtpool,
            tc.tile_pool(name="zt", bufs=2) as ztpool,
            tc.tile_pool(name="hsb", bufs=2) as hpool,
            tc.tile_pool(name="osb", bufs=2) as opool,
            tc.tile_pool(name="psw", bufs=WG + 1, space="PSUM") as pswpool,
            tc.tile_pool(name="psh", bufs=2, space="PSUM") as pshpool,
            tc.tile_pool(name="psp", bufs=1, space="PSUM") as psppool,
            tc.tile_pool(name="dram", bufs=1, space="DRAM") as dpool,
        ):
            # --- constants ---
            idx_sb = cpool.tile([128, NBLK * 8], i16)
            nc.sync.dma_start(out=idx_sb[:], in_=idx_d[:])
            slot_sb = cpool.tile([P, NBLK], f32)
            nc.sync.dma_start(out=slot_sb[:], in_=slot_d[:])
            ew_sb = cpool.tile([P, NBLK], f32)
            nc.sync.dma_start(out=ew_sb[:], in_=ew_d[:])
            iota_sb = cpool.tile([P, P], bf16)
            nc.sync.dma_start(out=iota_sb[:], in_=iota_d[:])
            ones_sb = cpool.tile([1, P], bf16)
            nc.sync.dma_start(out=ones_sb[:], in_=ones_d[:])
            w1_sb = cpool.tile([F, F], bf16)
            nc.sync.dma_start(out=w1_sb[:], in_=w1_d[:])
            w2_sb = cpool.tile([F, F], bf16)
            nc.sync.dma_start(out=w2_sb[:], in_=w2_d[:])
            b1_sb = cpool.tile([1, F], bf16)
            nc.sync.dma_start(out=b1_sb[:], in_=b1_d[:])
            b2_sb = cpool.tile([1, F], bf16)
            nc.sync.dma_start(out=b2_sb[:], in_=b2_d[:])
            batloc_sb = cpool.tile([P, NW], f32)
            nc.sync.dma_start(out=batloc_sb[:], in_=batloc_d[:])
            cnts_sb = cpool.tile([P, 1], f32)
            nc.sync.dma_start(out=cnts_sb[:], in_=cnts_d[:])

            # Funnel const-tile deps through the Vector engine (the ISA has a
            # small per-instruction sync-wait budget; same-engine ordering is
            # free).
            scratch = cpool.tile([P, 1], f32)
            for t in (slot_sb, ew_sb, iota_sb, w1_sb, w2_sb, batloc_sb, cnts_sb):
                nc.vector.reduce_sum(out=scratch[:], in_=t[:],
                                     axis=mybir.AxisListType.X)
            for t in (ones_sb, b1_sb, b2_sb):
                nc.vector.reduce_sum(out=scratch[:1, :], in_=t[:],
                                     axis=mybir.AxisListType.X)

            h1_shard = dpool.tile([NODE_PAD, F], bf16)
            h1_table = dpool.tile([TOT, F], bf16, addr_space="Shared")
            _local_tab = os.environ.get("KERNEL_LOCAL_TABLE", "0") == "1"
            _skip_l2 = os.environ.get("KERNEL_SKIP_L2", "0") == "1"
            if _local_tab:
                h1_local = dpool.tile([TOT, F], bf16)

            pool_ps = psppool.tile([P, F], f32)

            for layer in range(1 if _skip_l2 else 2):
                table = xt_d if layer == 0 else (
                    h1_local if _local_tab else h1_table)
                wmat_sb = w1_sb if layer == 0 else w2_sb
                b_sb = b1_sb if layer == 0 else b2_sb

                ps_tiles = {}
                for ci, (b0, nbk, q) in enumerate(calls):
                    g_t = gpool.tile([P, CALLBLK, P], bf16, tag="g")
                    nc.gpsimd.dma_gather(
                        out_ap=g_t[:, :nbk, :],
                        in_ap=table[q * QROWS:min((q + 1) * QROWS, TOT), :],
                        idxs_ap=idx_sb[:, b0 * 8:(b0 + nbk) * 8],
                        num_idxs=nbk * P,
                        num_idxs_reg=nbk * P,
                        elem_size=F,
                        queue_num=ci % _nq,
                    )
                    for j in range(nbk):
                        blk = b0 + j
                        w = int(blk_win[blk])
                        if blk_first[blk]:
                            ps_tiles[w] = pswpool.tile([P, P], f32, tag="psw", name=f"psw{w % 8}")
                        ps_w = ps_tiles[w]
                        mt = mtpool.tile([P, P], bf16, tag="mt")
                        nc.vector.tensor_scalar(
                            out=mt[:],
                            in0=iota_sb[:],
                            scalar1=slot_sb[:, blk:blk + 1],
                            scalar2=ew_sb[:, blk:blk + 1],
                            op0=mybir.AluOpType.is_equal,
                            op1=mybir.AluOpType.mult,
                        )
                        nc.tensor.matmul(
                            ps_w[:],
                            lhsT=g_t[:, j, :],
                            rhs=mt[:],
                            start=bool(blk_first[blk]),
                            stop=bool(blk_last[blk]),
                        )
                        if blk_last[blk]:
                            # ---- dense part for finished window w ----
                            zt = ztpool.tile([P, P], bf16, tag="zt")
                            nc.vector.tensor_copy(out=zt[:], in_=ps_w[:])
                            del ps_tiles[w]
                            ps_h = pshpool.tile([P, F], f32, tag="psh")
                            nc.tensor.matmul(
                                ps_h[:], lhsT=zt[:], rhs=wmat_sb[:],
                                start=True, stop=False,
                            )
                            nc.tensor.matmul(
                                ps_h[:], lhsT=ones_sb[:], rhs=b_sb[:],
                                start=False, stop=True,
                            )
                            h_sb = hpool.tile([P, F], bf16, tag="h")
                            nc.scalar.activation(
                                out=h_sb[:], in_=ps_h[:],
                                func=mybir.ActivationFunctionType.Relu,
                            )
                            if layer == 0:
                                nc.sync.dma_start(
                                    out=h1_shard[w * P:(w + 1) * P, :],
                                    in_=h_sb[:],
                                )
                            else:
                                mb = mtpool.tile([P, P], bf16, tag="mb")
                                nc.vector.tensor_scalar(
                                    out=mb[:],
                                    in0=iota_sb[:],
                                    scalar1=batloc_sb[:, w:w + 1],
                                    scalar2=None,
                                    op0=mybir.AluOpType.is_equal,
                                )
                                nc.tensor.matmul(
                                    pool_ps[:],
                                    lhsT=mb[:],
                                    rhs=h_sb[:],
                                    start=(w == 0),
                                    stop=(w == NW - 1),
                                )

                if layer == 0 and not _skip_l2:
                    nc.gpsimd.collective_compute(
                        "AllGather",
                        mybir.AluOpType.bypass,
                        replica_groups=[list(range(C))],
                        ins=[h1_shard[:]],
                        outs=[h1_table[:]],
                    )
                    if _local_tab:
                        nc.sync.dma_start(out=h1_local[:], in_=h1_table[:])

            # ---- finalize pool: divide by counts ----
            if _skip_l2:
                # touch pool_ps so it exists; output is meaningless
                nc.tensor.matmul(pool_ps[:], lhsT=ones_sb[:], rhs=b1_sb[:],
                                 start=True, stop=True)
            rec_sb = opool.tile([P, 1], f32, tag="rec")
            nc.vector.reciprocal(out=rec_sb[:], in_=cnts_sb[:])
            out_sb = opool.tile([P, F], f32, tag="os")
            nc.vector.tensor_scalar(
                out=out_sb[:],
                in0=pool_ps[:],
                scalar1=rec_sb[:, 0:1],
                scalar2=None,
                op0=mybir.AluOpType.mult,
            )
            nc.sync.dma_start(out=out_d[:], in_=out_sb[0:GPC, :])

    nc.compile()
    return nc


def kernel(x, edge_index, batch, W1, b1, W2, b2):
    x = np.asarray(x, np.float32)
    pre = _preprocess(x, edge_index, batch)

    iota = np.broadcast_to(np.arange(P, dtype=np.float32), (P, P)).astype(BF)
    ones = np.ones((1, P), BF)
    w1b = np.asarray(W1, np.float32).astype(BF)
    w2b = np.asarray(W2, np.float32).astype(BF)
    b1b = np.asarray(b1, np.float32).reshape(1, F).astype(BF)
    b2b = np.asarray(b2, np.float32).reshape(1, F).astype(BF)

    in_maps = []
    for c in range(C):
        in_maps.append({
            "xt": pre["xt"],
            "eidx": pre["idx_pc"][c],
            "eslot": pre["slot_pc"][c],
            "ew": pre["w_pc"][c],
            "iota": iota,
            "ones": ones,
            "w1": w1b,
            "w2": w2b,
            "b1": b1b,
            "b2": b2b,
            "batloc": pre["batloc_pc"][c],
            "cnts": pre["counts_pc"][c],
        })

    nc = _build_nc(pre)
    res = run_bass_kernel_spmd(nc, in_maps, core_ids=list(range(C)))
    out = np.concatenate([res.results[c]["out"] for c in range(C)], axis=0)
    return out.astype(np.float32)



# revision 2
# speedup vs baseline: 1.1415x; 1.1415x over previous
"""Trainium2 Bass kernel for a 2-layer GCN encoder + global mean pool.

Problem: x[100000,128] f32, edge_index[2,1600000] i64, batch[100000] i64
(sorted), W1/b1/W2/b2. Two GCNConv layers (symmetric deg^-1/2 norm, self
loops, relu) then mean-pool over 512 graphs -> [512, 128] f32.

Strategy (8 NeuronCores, data-parallel over graphs):
- Nodes partitioned by graph id into 8 contiguous shards (batch is sorted);
  each core owns the edges whose *destination* lands in its shard.
- Algebraic rewrite: aggregate-then-transform.  For each layer,
      h' = relu( (A_hat @ h) @ W + b ),   A_hat = D^-1/2 (A+I) D^-1/2
  so the sparse aggregation runs on raw 128-dim features and the dense
  matmul with W happens per 128-node window afterwards.  Self loops are
  plain edges with weight 1/deg.
- Sparse aggregation per core: per 128-edge block, dma_gather fetches the
  128 source rows (bf16, 256B each) from the feature table; a host-built
  selection matrix M_blk (one nonzero per edge row: its dst slot, valued
  with the GCN edge weight) is streamed from HBM as a plain contiguous
  DMA and scatters the gathered rows on the Tensor engine:
      PSUM[f, s] += G_blk.T @ M_blk
  accumulated over all blocks of a 128-node destination window.  The M
  matrices are identical for both layers (same graph), so one table in
  HBM serves both.
- dma_gather uses int16 indices, so the node table is addressed through
  <=32768-row "quarter" slices; edges are sorted by (window-group, quarter,
  window) so each gather call stays quarter-pure while the PSUM tiles of a
  4-window group stay resident across the group's quarter runs.
- The finished PSUM window (= Z^T) feeds the dense W matmul directly (its
  transposed layout is exactly the lhsT the Tensor engine wants); bias is
  added via a K=1 matmul with a ones row; relu on the Scalar engine.
- Between layers one 8-core AllGather shares the per-shard h1 table (bf16).
- Mean pool: host-built one-hot graph-slot matrices (streamed once) feed
  per-window matmuls into a single resident PSUM bank, then a
  reciprocal-count scale.

Host-side preprocessing is purely structural (index sorting, degree counts,
normalization coefficients and one-hot selection matrices derived from the
graph topology); all feature/weight compute runs on device.
"""

import math
import os

import numpy as np
import ml_dtypes

import concourse.bass as bass
import concourse.bacc as bacc
import concourse.mybir as mybir
import concourse.tile as tile
from concourse.bass_utils import run_bass_kernel_spmd

P = 128
C = 8               # cores
G = 512             # graphs
GPC = G // C        # graphs per core
F = 128             # feature dim (in = hid = out)
WG = 4              # windows per PSUM-resident group
QROWS = 32768       # rows per int16-addressable table slice

bf16 = mybir.dt.bfloat16
f32 = mybir.dt.float32
i16 = mybir.dt.int16

BF = ml_dtypes.bfloat16

CALLBLK = int(os.environ.get("KERNEL_CALLBLK", "8"))


def _preprocess(x, edge_index, batch):
    """Structural preprocessing: shard nodes by graph, sort/pad edges by
    (window-group, src-quarter, dst-window), compute GCN norm weights and
    host-built selection matrices."""
    N = x.shape[0]
    src = np.asarray(edge_index[0], dtype=np.int64)
    dst = np.asarray(edge_index[1], dtype=np.int64)
    batch = np.asarray(batch, dtype=np.int64)

    node_start = np.searchsorted(batch, np.arange(C + 1) * GPC).astype(np.int64)
    nk = np.diff(node_start)
    NODE_PAD = int(math.ceil(nk.max() / P) * P)
    NW = NODE_PAD // P
    TOT = C * NODE_PAD
    assert TOT <= 4 * QROWS
    NG = (NW + WG - 1) // WG

    core_of = (batch // GPC).astype(np.int64)
    row = (np.arange(N) - node_start[core_of] + core_of * NODE_PAD).astype(np.int64)

    deg = np.bincount(dst, minlength=N).astype(np.float64) + 1.0
    dis = 1.0 / np.sqrt(deg)

    # full edge list including self loops
    esrc = np.concatenate([src, np.arange(N)])
    edst = np.concatenate([dst, np.arange(N)])
    ew = np.concatenate([dis[src] * dis[dst], 1.0 / deg]).astype(np.float32)

    ecore = core_of[edst]
    eld = edst - node_start[ecore]
    ewin = eld >> 7
    eslot = (eld & 127).astype(np.int64)
    esrcrow = row[esrc]
    eq = (esrcrow // QROWS).astype(np.int64)      # source quarter
    eloc = (esrcrow % QROWS).astype(np.int64)     # quarter-local row
    egrp = ewin // WG

    NQ = 4
    # segment id in (core, group, quarter, window) order
    seg = ((ecore * NG + egrp) * NQ + eq) * NW + ewin
    NSEG = C * NG * NQ * NW
    counts = np.bincount(seg, minlength=NSEG)
    cnt4 = counts.reshape(C, NG, NQ, NW)
    # SPMD-common block counts per (group, quarter, window)
    BWS = np.ceil(cnt4.max(axis=0) / P).astype(np.int64)      # [NG, NQ, NW]
    for g in range(NG):
        mask = np.zeros(NW, bool)
        mask[g * WG:(g + 1) * WG] = True
        BWS[g, :, ~mask] = 0
    # every window needs >= 1 block so its PSUM tile is always written
    for w in range(NW):
        g = w // WG
        if BWS[g, :, w].sum() == 0:
            BWS[g, 0, w] = 1
    NBLK = int(BWS.sum())

    # block/segment offsets in (g, q, w) order
    seg_order = []          # (g, q, w, block_start, nblocks)
    seg_start = np.zeros((NG, NQ, NW), np.int64)
    acc = 0
    for g in range(NG):
        for q in range(NQ):
            for w in range(g * WG, min((g + 1) * WG, NW)):
                seg_start[g, q, w] = acc
                nb = int(BWS[g, q, w])
                if nb:
                    seg_order.append((g, q, w, acc, nb))
                acc += nb
    assert acc == NBLK

    # scatter edges into the padded per-core layout
    order = np.argsort(seg, kind="stable")
    seg_sorted = seg[order]
    grp_excl = np.concatenate([[0], np.cumsum(counts)[:-1]])
    pos = np.arange(order.size) - grp_excl[seg_sorted]
    es = order
    dest = (ecore[es] * (NBLK * P)
            + seg_start[egrp[es], eq[es], ewin[es]] * P + pos)

    idx_arr = np.zeros(C * NBLK * P, np.int16)      # quarter-local src row
    slot_arr = np.zeros(C * NBLK * P, np.int64)
    w_arr = np.zeros(C * NBLK * P, np.float32)
    idx_arr[dest] = eloc[es].astype(np.int16)
    slot_arr[dest] = eslot[es]
    w_arr[dest] = ew[es]

    # per-core uploads
    # idx: wrapped [16, NBLK*8] (logical i at [i%16, i//16]), replicated to
    # 128 partitions (the gather ucode's per-Q7-core channel groups all read
    # the same wrap)
    idx_pc = np.ascontiguousarray(
        idx_arr.reshape(C, NBLK * P // 16, 16).transpose(0, 2, 1))
    idx_pc = np.ascontiguousarray(np.tile(idx_pc, (1, 8, 1)))

    # host-built selection matrices, transposed layout [128, NBLK*128]:
    # partition p holds, for each block b, the row M_b[p, :] (one nonzero
    # at the dst slot of the edge in lane p of block b).
    m_pc = np.zeros((C, P, NBLK * P), BF)
    sl = slot_arr.reshape(C, NBLK, P)
    wv = w_arr.reshape(C, NBLK, P)
    for c in range(C):
        m = np.zeros((NBLK, P, P), np.float32)
        np.put_along_axis(m, sl[c][:, :, None], wv[c][:, :, None], axis=2)
        m_pc[c] = m.transpose(1, 0, 2).reshape(P, NBLK * P).astype(BF)

    # node feature table, padded/bf16
    xt = np.zeros((TOT, F), BF)
    xt[row] = np.asarray(x, np.float32).astype(BF)

    # static schedule: per block -> (window, first/last-of-window);
    # gather calls: chunks of <= CALLBLK blocks within one (g, q) run.
    blk_win = np.zeros(NBLK, np.int64)
    first_blk = {}
    last_blk = {}
    for (g, q, w, b0, nb) in seg_order:
        blk_win[b0:b0 + nb] = w
        if w not in first_blk:
            first_blk[w] = b0
        last_blk[w] = b0 + nb - 1
    blk_first = np.zeros(NBLK, bool)
    blk_last = np.zeros(NBLK, bool)
    for w, b in first_blk.items():
        blk_first[b] = True
    for w, b in last_blk.items():
        blk_last[b] = True

    calls = []   # (b0, nb, quarter)
    run_key = None
    run_blocks = []
    runs = []
    for (g, q, w, b0, nb) in seg_order:
        if (g, q) != run_key:
            if run_blocks:
                runs.append((run_key[1], run_blocks))
            run_key = (g, q)
            run_blocks = []
        run_blocks.append((b0, nb))
    if run_blocks:
        runs.append((run_key[1], run_blocks))
    for q, blocks in runs:
        b0 = blocks[0][0]
        bend = blocks[-1][0] + blocks[-1][1]
        b = b0
        while b < bend:
            nb = min(CALLBLK, bend - b)
            calls.append((b, nb, q))
            b += nb

    # pooling metadata: host-built one-hot graph-slot matrices
    # [128, NW*128]: partition p holds, for window w, onehot(graph-slot of
    # node w*128+p).
    poolm_pc = np.zeros((C, P, NW * P), BF)
    for c in range(C):
        batloc = np.full(NODE_PAD, -1, np.int64)
        nn = int(nk[c])
        batloc[:nn] = batch[node_start[c]:node_start[c + 1]] - c * GPC
        pm = np.zeros((NW, P, P), np.float32)
        valid = batloc >= 0
        bl2 = batloc.reshape(NW, P)
        v2 = valid.reshape(NW, P)
        for w in range(NW):
            pm[w, v2[w], bl2[w][v2[w]]] = 1.0
        poolm_pc[c] = pm.transpose(1, 0, 2).reshape(P, NW * P).astype(BF)

    gcnt = np.bincount(batch, minlength=G).astype(np.float32)
    counts_pc = np.ones((C, P, 1), np.float32)
    counts_pc[:, :GPC, 0] = gcnt.reshape(C, GPC)

    return dict(
        NODE_PAD=NODE_PAD, NW=NW, TOT=TOT, NBLK=NBLK,
        blk_win=blk_win, blk_first=blk_first, blk_last=blk_last, calls=calls,
        idx_pc=idx_pc, m_pc=m_pc, xt=xt,
        poolm_pc=poolm_pc, counts_pc=counts_pc,
    )


def _build_nc(pre):
    NW = pre["NW"]
    NBLK = pre["NBLK"]
    TOT = pre["TOT"]
    NODE_PAD = pre["NODE_PAD"]
    blk_win = pre["blk_win"]
    blk_first = pre["blk_first"]
    blk_last = pre["blk_last"]
    calls = pre["calls"]

    _nq = int(os.environ.get("KERNEL_NQUEUES", "4"))
    nc = bacc.Bacc(None, num_devices=C, num_swdge_queues=_nq)

    xt_d = nc.dram_tensor("xt", [TOT, F], bf16, kind="ExternalInput")
    idx_d = nc.dram_tensor("eidx", [128, NBLK * 8], i16, kind="ExternalInput")
    m_d = nc.dram_tensor("emat", [P, NBLK * P], bf16, kind="ExternalInput")
    ones_d = nc.dram_tensor("ones", [1, P], bf16, kind="ExternalInput")
    w1_d = nc.dram_tensor("w1", [F, F], bf16, kind="ExternalInput")
    w2_d = nc.dram_tensor("w2", [F, F], bf16, kind="ExternalInput")
    b1_d = nc.dram_tensor("b1", [1, F], bf16, kind="ExternalInput")
    b2_d = nc.dram_tensor("b2", [1, F], bf16, kind="ExternalInput")
    poolm_d = nc.dram_tensor("poolm", [P, NW * P], bf16, kind="ExternalInput")
    cnts_d = nc.dram_tensor("cnts", [P, 1], f32, kind="ExternalInput")
    out_d = nc.dram_tensor("out", [GPC, F], f32, kind="ExternalOutput")

    with tile.TileContext(nc) as tc:
        with (
            tc.tile_pool(name="const", bufs=1) as cpool,
            tc.tile_pool(name="gbuf", bufs=6) as gpool,
            tc.tile_pool(name="mbuf", bufs=6) as mpool,
            tc.tile_pool(name="zt", bufs=2) as ztpool,
            tc.tile_pool(name="hsb", bufs=2) as hpool,
            tc.tile_pool(name="osb", bufs=2) as opool,
            tc.tile_pool(name="psw", bufs=WG + 1, space="PSUM") as pswpool,
            tc.tile_pool(name="psh", bufs=2, space="PSUM") as pshpool,
            tc.tile_pool(name="psp", bufs=1, space="PSUM") as psppool,
            tc.tile_pool(name="dram", bufs=1, space="DRAM") as dpool,
        ):
            # --- constants ---
            idx_sb = cpool.tile([128, NBLK * 8], i16)
            nc.sync.dma_start(out=idx_sb[:], in_=idx_d[:])
            ones_sb = cpool.tile([1, P], bf16)
            nc.sync.dma_start(out=ones_sb[:], in_=ones_d[:])
            w1_sb = cpool.tile([F, F], bf16)
            nc.sync.dma_start(out=w1_sb[:], in_=w1_d[:])
            w2_sb = cpool.tile([F, F], bf16)
            nc.sync.dma_start(out=w2_sb[:], in_=w2_d[:])
            b1_sb = cpool.tile([1, F], bf16)
            nc.sync.dma_start(out=b1_sb[:], in_=b1_d[:])
            b2_sb = cpool.tile([1, F], bf16)
            nc.sync.dma_start(out=b2_sb[:], in_=b2_d[:])
            poolm_sb = cpool.tile([P, NW * P], bf16)
            nc.sync.dma_start(out=poolm_sb[:], in_=poolm_d[:])
            cnts_sb = cpool.tile([P, 1], f32)
            nc.sync.dma_start(out=cnts_sb[:], in_=cnts_d[:])

            # Funnel const-tile deps through the Vector engine (the ISA has a
            # small per-instruction sync-wait budget; same-engine ordering is
            # free).
            scratch = cpool.tile([P, 1], f32)
            for t in (w1_sb, w2_sb, poolm_sb, cnts_sb):
                nc.vector.reduce_sum(out=scratch[:], in_=t[:],
                                     axis=mybir.AxisListType.X)
            for t in (ones_sb, b1_sb, b2_sb):
                nc.vector.reduce_sum(out=scratch[:1, :], in_=t[:],
                                     axis=mybir.AxisListType.X)

            h1_shard = dpool.tile([NODE_PAD, F], bf16)
            h1_table = dpool.tile([TOT, F], bf16, addr_space="Shared")
            _skip_l2 = os.environ.get("KERNEL_SKIP_L2", "0") == "1"

            pool_ps = psppool.tile([P, F], f32)

            for layer in range(1 if _skip_l2 else 2):
                table = xt_d if layer == 0 else h1_table
                wmat_sb = w1_sb if layer == 0 else w2_sb
                b_sb = b1_sb if layer == 0 else b2_sb

                ps_tiles = {}
                for ci, (b0, nbk, q) in enumerate(calls):
                    g_t = gpool.tile([P, CALLBLK, P], bf16, tag="g")
                    nc.gpsimd.dma_gather(
                        out_ap=g_t[:, :nbk, :],
                        in_ap=table[q * QROWS:min((q + 1) * QROWS, TOT), :],
                        idxs_ap=idx_sb[:, b0 * 8:(b0 + nbk) * 8],
                        num_idxs=nbk * P,
                        num_idxs_reg=nbk * P,
                        elem_size=F,
                        queue_num=ci % _nq,
                    )
                    m_t = mpool.tile([P, CALLBLK * P], bf16, tag="m")
                    nc.sync.dma_start(
                        out=m_t[:, :nbk * P],
                        in_=m_d[:, b0 * P:(b0 + nbk) * P],
                    )
                    for j in range(nbk):
                        blk = b0 + j
                        w = int(blk_win[blk])
                        if blk_first[blk]:
                            ps_tiles[w] = pswpool.tile(
                                [P, P], f32, tag="psw", name=f"psw{w % 8}")
                        ps_w = ps_tiles[w]
                        nc.tensor.matmul(
                            ps_w[:],
                            lhsT=g_t[:, j, :],
                            rhs=m_t[:, j * P:(j + 1) * P],
                            start=bool(blk_first[blk]),
                            stop=bool(blk_last[blk]),
                        )
                        if blk_last[blk]:
                            # ---- dense part for finished window w ----
                            zt = ztpool.tile([P, P], bf16, tag="zt")
                            nc.vector.tensor_copy(out=zt[:], in_=ps_w[:])
                            del ps_tiles[w]
                            ps_h = pshpool.tile([P, F], f32, tag="psh")
                            nc.tensor.matmul(
                                ps_h[:], lhsT=zt[:], rhs=wmat_sb[:],
                                start=True, stop=False,
                            )
                            nc.tensor.matmul(
                                ps_h[:], lhsT=ones_sb[:], rhs=b_sb[:],
                                start=False, stop=True,
                            )
                            h_sb = hpool.tile([P, F], bf16, tag="h")
                            nc.scalar.activation(
                                out=h_sb[:], in_=ps_h[:],
                                func=mybir.ActivationFunctionType.Relu,
                            )
                            if layer == 0:
                                nc.sync.dma_start(
                                    out=h1_shard[w * P:(w + 1) * P, :],
                                    in_=h_sb[:],
                                )
                            else:
                                nc.tensor.matmul(
                                    pool_ps[:],
                                    lhsT=poolm_sb[:, w * P:(w + 1) * P],
                                    rhs=h_sb[:],
                                    start=(w == 0),
                                    stop=(w == NW - 1),
                                )

                if layer == 0 and not _skip_l2:
                    nc.gpsimd.collective_compute(
                        "AllGather",
                        mybir.AluOpType.bypass,
                        replica_groups=[list(range(C))],
                        ins=[h1_shard[:]],
                        outs=[h1_table[:]],
                    )

            # ---- finalize pool: divide by counts ----
            if _skip_l2:
                # touch pool_ps so it exists; output is meaningless
                nc.tensor.matmul(pool_ps[:], lhsT=ones_sb[:], rhs=b1_sb[:],
                                 start=True, stop=True)
            rec_sb = opool.tile([P, 1], f32, tag="rec")
            nc.vector.reciprocal(out=rec_sb[:], in_=cnts_sb[:])
            out_sb = opool.tile([P, F], f32, tag="os")
            nc.vector.tensor_scalar(
                out=out_sb[:],
                in0=pool_ps[:],
                scalar1=rec_sb[:, 0:1],
                scalar2=None,
                op0=mybir.AluOpType.mult,
            )
            nc.sync.dma_start(out=out_d[:], in_=out_sb[0:GPC, :])

    nc.compile()
    return nc


def kernel(x, edge_index, batch, W1, b1, W2, b2):
    x = np.asarray(x, np.float32)
    pre = _preprocess(x, edge_index, batch)

    ones = np.ones((1, P), BF)
    w1b = np.asarray(W1, np.float32).astype(BF)
    w2b = np.asarray(W2, np.float32).astype(BF)
    b1b = np.asarray(b1, np.float32).reshape(1, F).astype(BF)
    b2b = np.asarray(b2, np.float32).reshape(1, F).astype(BF)

    in_maps = []
    for c in range(C):
        in_maps.append({
            "xt": pre["xt"],
            "eidx": pre["idx_pc"][c],
            "emat": pre["m_pc"][c],
            "ones": ones,
            "w1": w1b,
            "w2": w2b,
            "b1": b1b,
            "b2": b2b,
            "poolm": pre["poolm_pc"][c],
            "cnts": pre["counts_pc"][c],
        })

    nc = _build_nc(pre)
    res = run_bass_kernel_spmd(nc, in_maps, core_ids=list(range(C)))
    out = np.concatenate([res.results[c]["out"] for c in range(C)], axis=0)
    return out.astype(np.float32)
